# revision 1
# baseline (speedup 1.0000x reference)
"""LSTMCell Trainium2 kernel.

Full-input contract: kernel(**inputs) takes the complete (16384, 1024) fp32
tensors, shards the batch dim across 8 NeuronCores (data-parallel, weights
replicated), runs a Bass/Tile kernel per core, and gathers (h, c).

Per-core plan (B_local = 2048):
  - x/h are cast fp32->fp16 during the SWDGE DMA load, then transposed with
    the HWDGE xbar into k-partitioned [128, 128] tiles (contraction dim on
    partitions).
  - W (8 matrices) is pre-concatenated on the host into one [2048, 4096] fp32
    matrix (rows: x-weights then h-weights; cols: gates [i|f|o|u]) and kept
    fp16-resident in SBUF (cast during DMA load).
  - For each of 16 m-tiles (128 batch rows): 8 PSUM banks accumulate
    preact[:, s*512:(s+1)*512] over 16 k-tiles; DVE adds the (partition-
    broadcast) bias; ScalarE applies Sigmoid/Tanh; VectorE combines
    c' = f*c + i*u, h' = o*tanh(c'); results DMA out as fp32.
"""

import sys

if "/opt/trn_rl_repo" not in sys.path:
    sys.path.insert(0, "/opt/trn_rl_repo")

import numpy as np

import concourse.bass as bass  # noqa: F401
import concourse.mybir as mybir
import concourse.tile as tile
from concourse import bacc
from concourse.bass_utils import run_bass_kernel_spmd

F32 = mybir.dt.float32
F16 = mybir.dt.float16

N_CORES = 8
B_FULL = 16384
IN = 1024
H = 1024
B_LOCAL = B_FULL // N_CORES  # 2048
P = 128
K_TILES = (IN + H) // P      # 16
N_TOTAL = 4 * H              # 4096 (gates i|f|o|u)
N_SLICES = N_TOTAL // 512    # 8
SIG = mybir.ActivationFunctionType.Sigmoid
TANH = mybir.ActivationFunctionType.Tanh
ADD = mybir.AluOpType.add
MULT = mybir.AluOpType.mult


class _NullCtx:
    def __enter__(self):
        return None

    def __exit__(self, *a):
        return False


def _maybe_for_i(tc, reps):
    return tc.For_i(0, reps, 1) if reps > 1 else _NullCtx()


def build_nc(b_local: int = B_LOCAL, reps: int = 1, loop_order: str = "smajor"):
    """reps > 1 wraps the whole body in a For_i that recomputes the same
    outputs; used only by test.py to time the kernel body on hardware
    (dispatch overhead over the axon tunnel is ~100ms, so a single body
    can't be wall-clocked).

    loop_order:
      smajor  - per m-tile: for each 512-col slice, run all 16 k matmuls
                (stationary reloaded every matmul, psum freed slice by slice)
      kmajor  - per m-tile: for each k, run all 8 slices (stationary reused
                across 8 matmuls, all 8 psum banks held to the end)
      kmajor4 - kmajor over two groups of 4 slices
    """
    m_tiles = b_local // P
    nc = bacc.Bacc("TRN2", target_bir_lowering=False, debug=False)

    x_d = nc.dram_tensor("x", [b_local, IN], F32, kind="ExternalInput")
    h_d = nc.dram_tensor("h", [b_local, H], F32, kind="ExternalInput")
    c_d = nc.dram_tensor("c", [b_local, H], F32, kind="ExternalInput")
    w_d = nc.dram_tensor("w", [IN + H, N_TOTAL], F32, kind="ExternalInput")
    b_d = nc.dram_tensor("b", [N_TOTAL], F32, kind="ExternalInput")
    ho_d = nc.dram_tensor("h_out", [b_local, H], F32, kind="ExternalOutput")
    co_d = nc.dram_tensor("c_out", [b_local, H], F32, kind="ExternalOutput")

    with tile.TileContext(nc) as tc:
        with (
            tc.tile_pool(name="wpool", bufs=1) as wpool,
            tc.tile_pool(name="const", bufs=1) as const,
        ):
            # Resident fp16 weights, k on partitions: [128, kt, 4096]
            w16 = wpool.tile([P, K_TILES, N_TOTAL], F16)
            for kt in range(K_TILES):
                nc.gpsimd.dma_start(w16[:, kt, :], w_d.ap()[kt * P : (kt + 1) * P, :])

            # Bias broadcast across partitions: [128, 4096] fp32.  The staging
            # tile lives in a pool that closes before the main pools open
            # (SBUF is tight: weights take 128KB/partition).
            bb = const.tile([P, N_TOTAL], F32)
            with tc.tile_pool(name="binit", bufs=1) as binit:
                b_sb = binit.tile([1, N_TOTAL], F32)
                nc.sync.dma_start(b_sb[:], b_d.ap().rearrange("(o n) -> o n", o=1))
                nc.gpsimd.partition_broadcast(bb[:], b_sb[:])

            with (
                tc.tile_pool(name="stage", bufs=2) as stage,
                tc.tile_pool(name="xt", bufs=2) as xtp,
                tc.tile_pool(name="cin", bufs=2) as cin,
                tc.tile_pool(name="gate", bufs=2) as gp,
                tc.tile_pool(name="tmp", bufs=2) as tp,
                tc.tile_pool(
                    name="ps",
                    bufs={"smajor": 8, "kmajor": 1, "kmajor4": 2}[loop_order],
                    space="PSUM",
                ) as ps,
                _maybe_for_i(tc, reps),
            ):
                for m in range(m_tiles):
                    rows = slice(m * P, (m + 1) * P)
                    x16 = stage.tile([P, IN], F16, tag="x16")
                    nc.gpsimd.dma_start(x16[:], x_d.ap()[rows, :])
                    h16 = stage.tile([P, H], F16, tag="h16")
                    nc.gpsimd.dma_start(h16[:], h_d.ap()[rows, :])

                    xhT = xtp.tile([P, K_TILES, P], F16, tag="xhT")
                    for kt in range(IN // P):
                        nc.sync.dma_start(
                            xhT[:, kt, :], x16[:, kt * P : (kt + 1) * P], transpose=True
                        )
                    for kt in range(H // P):
                        nc.sync.dma_start(
                            xhT[:, IN // P + kt, :],
                            h16[:, kt * P : (kt + 1) * P],
                            transpose=True,
                        )

                    cprev = cin.tile([P, H], F32, tag="cprev")
                    nc.scalar.dma_start(cprev[:], c_d.ap()[rows, :])

                    gates = gp.tile([P, N_TOTAL], F16, tag="gates")

                    def drain_slice(s, pt):
                        sl = slice(s * 512, (s + 1) * 512)
                        nc.vector.tensor_tensor(gates[:, sl], pt[:], bb[:, sl], ADD)
                        nc.scalar.activation(
                            gates[:, sl], gates[:, sl], TANH if s >= 6 else SIG
                        )

                    if loop_order == "smajor":
                        for s in range(N_SLICES):
                            pt = ps.tile([P, 512], F32, tag="psum")
                            for kt in range(K_TILES):
                                nc.tensor.matmul(
                                    pt[:],
                                    lhsT=xhT[:, kt, :],
                                    rhs=w16[:, kt, s * 512 : (s + 1) * 512],
                                    start=(kt == 0),
                                    stop=(kt == K_TILES - 1),
                                )
                            drain_slice(s, pt)
                    else:
                        group = N_SLICES if loop_order == "kmajor" else 4
                        for g0 in range(0, N_SLICES, group):
                            pts = [
                                ps.tile([P, 512], F32, tag=f"psum{si}", name=f"pt{si}")
                                for si in range(group)
                            ]
                            for kt in range(K_TILES):
                                for si in range(group):
                                    s = g0 + si
                                    nc.tensor.matmul(
                                        pts[si][:],
                                        lhsT=xhT[:, kt, :],
                                        rhs=w16[:, kt, s * 512 : (s + 1) * 512],
                                        start=(kt == 0),
                                        stop=(kt == K_TILES - 1),
                                    )
                            for si in range(group):
                                drain_slice(g0 + si, pts[si])

                    i_g = gates[:, 0:H]
                    f_g = gates[:, H : 2 * H]
                    o_g = gates[:, 2 * H : 3 * H]
                    u_g = gates[:, 3 * H : 4 * H]

                    t1 = tp.tile([P, H], F32, tag="t1")
                    nc.vector.tensor_tensor(t1[:], f_g, cprev[:], MULT)
                    t2 = tp.tile([P, H], F32, tag="t2")
                    nc.vector.tensor_tensor(t2[:], i_g, u_g, MULT)
                    # c' overwrites the cprev slot; tanh(c') goes to t1's
                    # slot; h' to t2's slot.
                    nc.vector.tensor_tensor(cprev[:], t1[:], t2[:], ADD)
                    nc.scalar.activation(t1[:], cprev[:], TANH)
                    nc.vector.tensor_tensor(t2[:], o_g, t1[:], MULT)

                    nc.scalar.dma_start(co_d.ap()[rows, :], cprev[:])
                    nc.scalar.dma_start(ho_d.ap()[rows, :], t2[:])

    nc.compile()
    return nc


_NC_CACHE: dict = {}


def _get_nc(b_local: int = B_LOCAL):
    if b_local not in _NC_CACHE:
        _NC_CACHE[b_local] = build_nc(b_local)
    return _NC_CACHE[b_local]


def make_in_maps(
    input, prev_h, prev_c,
    weight_xi, weight_hi, weight_xf, weight_hf,
    weight_xu, weight_hu, weight_xo, weight_ho,
    bias_i, bias_f, bias_o, bias_u,
):
    """Host-side shard/pack: batch split across cores, weights replicated."""
    asnp = lambda a: np.ascontiguousarray(np.asarray(a, dtype=np.float32))
    # Gate column order [i | f | o | u]; K rows: x-weights then h-weights.
    w_cat = np.concatenate(
        [
            np.concatenate([asnp(weight_xi), asnp(weight_xf), asnp(weight_xo), asnp(weight_xu)], axis=1),
            np.concatenate([asnp(weight_hi), asnp(weight_hf), asnp(weight_ho), asnp(weight_hu)], axis=1),
        ],
        axis=0,
    )
    b_cat = np.concatenate([asnp(bias_i), asnp(bias_f), asnp(bias_o), asnp(bias_u)], axis=0)
    x = asnp(input)
    hh = asnp(prev_h)
    cc = asnp(prev_c)
    in_maps = []
    for core in range(N_CORES):
        r = slice(core * B_LOCAL, (core + 1) * B_LOCAL)
        in_maps.append({"x": x[r], "h": hh[r], "c": cc[r], "w": w_cat, "b": b_cat})
    return in_maps


def kernel(**inputs):
    nc = _get_nc()
    in_maps = make_in_maps(**inputs)
    res = run_bass_kernel_spmd(nc, in_maps, core_ids=list(range(N_CORES)))
    h_full = np.concatenate([res.results[c]["h_out"] for c in range(N_CORES)], axis=0)
    c_full = np.concatenate([res.results[c]["c_out"] for c in range(N_CORES)], axis=0)
    return (h_full, c_full)


if __name__ == "__main__":
    rng = np.random.default_rng(0)
    stdv = 1.0 / np.sqrt(H)
    ins = {
        "input": rng.standard_normal((B_FULL, IN), dtype=np.float32),
        "prev_h": rng.standard_normal((B_FULL, H), dtype=np.float32),
        "prev_c": rng.standard_normal((B_FULL, H), dtype=np.float32),
    }
    for nm in ["weight_xi", "weight_hi", "weight_xf", "weight_hf",
               "weight_xu", "weight_hu", "weight_xo", "weight_ho"]:
        ins[nm] = rng.uniform(-stdv, stdv, (IN, H)).astype(np.float32)
    for nm in ["bias_i", "bias_f", "bias_o", "bias_u"]:
        ins[nm] = rng.uniform(-stdv, stdv, (H,)).astype(np.float32)
    h, c = kernel(**ins)
    print("kernel ran:", h.shape, c.shape)



# revision 2
# speedup vs baseline: 38.6608x; 38.6608x over previous
"""LSTMCell Trainium2 kernel.

Full-input contract: kernel(**inputs) takes the complete (16384, 1024) fp32
tensors, shards the batch dim across 8 NeuronCores (data-parallel, weights
replicated), runs a Bass/Tile kernel per core, and gathers (h, c).

Per-core plan (B_local = 2048):
  - Inputs packed host-side: xhc = [x; prev_h; prev_c] rows ([3*B_local,
    1024] fp32), w = fp16 [2048, 4096] (x-weight rows then h-weight rows,
    gate cols [i|f|o|u]), b = fp32 [4096].  Output hc = [h'; c'] rows.
  - Weights are fetched as 16 per-k-tile HWDGE DMAs into a resident fp16
    SBUF tile [128, kt=16, 4096] (k on partitions), so the first matmuls
    can start before the whole 16 MB lands.
  - Per m-tile (128 batch rows): ONE gpsimd DMA loads the x+h rows with an
    fp32->fp16 cast; ONE HWDGE xbar-transpose instruction produces all 16
    k-partitioned tiles (xhT[p, kt, q] = xh16[q, kt*128+p]); 8 PSUM banks
    each accumulate preact[:, s*512:(s+1)*512] over the 16 k-tiles; DVE
    adds the partition-broadcast bias; ScalarE applies Sigmoid/Tanh;
    VectorE combines c' = f*c + i*u, h' = o*tanh(c'); h'/c' DMA out fp32.

reps > 1 wraps the whole body -- weight fetch included -- in a hardware
For_i that recomputes the same outputs.  test.py uses it to measure the
sustained per-execution time on hardware (a single execution cannot be
wall-clocked over the axon tunnel: ~70 ms dispatch latency vs ~0.5 ms of
device work).
"""

import sys

if "/opt/trn_rl_repo" not in sys.path:
    sys.path.insert(0, "/opt/trn_rl_repo")

import numpy as np

import concourse.bass as bass  # noqa: F401
import concourse.mybir as mybir
import concourse.tile as tile
from concourse import bacc
from concourse.bass_utils import run_bass_kernel_spmd

F32 = mybir.dt.float32
F16 = mybir.dt.float16

N_CORES = 8
B_FULL = 16384
IN = 1024
H = 1024
B_LOCAL = B_FULL // N_CORES  # 2048
P = 128
K_TILES = (IN + H) // P      # 16
N_TOTAL = 4 * H              # 4096 (gates i|f|o|u)
N_SLICES = N_TOTAL // 512    # 8
SIG = mybir.ActivationFunctionType.Sigmoid
TANH = mybir.ActivationFunctionType.Tanh
ADD = mybir.AluOpType.add
MULT = mybir.AluOpType.mult


class _NullCtx:
    def __enter__(self):
        return None

    def __exit__(self, *a):
        return False


def _maybe_for_i(tc, reps):
    return tc.For_i(0, reps, 1) if reps > 1 else _NullCtx()


def build_nc(b_local: int = B_LOCAL, reps: int = 1):
    m_tiles = b_local // P
    nc = bacc.Bacc("TRN2", target_bir_lowering=False, debug=False)

    xhc_d = nc.dram_tensor("xhc", [3 * b_local, IN], F32, kind="ExternalInput")
    w_d = nc.dram_tensor("w", [IN + H, N_TOTAL], F16, kind="ExternalInput")
    b_d = nc.dram_tensor("b", [N_TOTAL], F32, kind="ExternalInput")
    hc_d = nc.dram_tensor("hc_out", [2 * b_local, H], F32, kind="ExternalOutput")

    xhc_v = xhc_d.ap().rearrange("(pl b) f -> b pl f", pl=3)
    hc_v = hc_d.ap().rearrange("(pl b) f -> b pl f", pl=2)
    w_v = w_d.ap().rearrange("(kt p) g -> p kt g", kt=K_TILES)

    with tile.TileContext(nc) as tc:
        with (
            tc.tile_pool(name="wpool", bufs=1) as wpool,
            tc.tile_pool(name="const", bufs=1) as const,
        ):
            # Bias broadcast across partitions: [128, 4096] fp32.  The
            # staging tile lives in a pool that closes before the main
            # pools open (SBUF is tight: weights take 128KB/partition).
            bb = const.tile([P, N_TOTAL], F32)
            with tc.tile_pool(name="binit", bufs=1) as binit:
                b_sb = binit.tile([1, N_TOTAL], F32)
                nc.sync.dma_start(b_sb[:], b_d.ap().rearrange("(o n) -> o n", o=1))
                nc.gpsimd.partition_broadcast(bb[:], b_sb[:])

            # Resident fp16 weights, k on partitions: [128, kt, 4096]
            w16 = wpool.tile([P, K_TILES, N_TOTAL], F16)

            with (
                tc.tile_pool(name="stage", bufs=2) as stage,
                tc.tile_pool(name="xt", bufs=2) as xtp,
                tc.tile_pool(name="cin", bufs=2) as cin,
                tc.tile_pool(name="gate", bufs=2) as gp,
                tc.tile_pool(name="tmp", bufs=2) as tp,
                tc.tile_pool(name="ps", bufs=8, space="PSUM") as ps,
                _maybe_for_i(tc, reps),
            ):
                # Weight fetch (fp16, HWDGE, no cast) inside the reps loop
                # so each rep pays the full kernel's HBM traffic.
                for kt in range(K_TILES):
                    nc.sync.dma_start(w16[:, kt, :], w_v[:, kt, :])

                for m in range(m_tiles):
                    rows = slice(m * P, (m + 1) * P)
                    # x+h rows for this m-tile in one DMA, fp32->fp16 cast.
                    xh16 = stage.tile([P, 2, IN], F16, tag="xh16")
                    nc.gpsimd.dma_start(xh16[:], xhc_v[rows, 0:2, :])
                    # All 16 k-tiles transposed in one xbar instruction:
                    # xhT[p, kt, q] = xh16[q, kt*128+p]
                    xhT = xtp.tile([P, K_TILES, P], F16, tag="xhT")
                    nc.sync.dma_start(xhT[:], xh16[:], transpose=True)

                    cprev = cin.tile([P, H], F32, tag="cprev")
                    nc.scalar.dma_start(cprev[:], xhc_v[rows, 2, :])

                    gates = gp.tile([P, N_TOTAL], F16, tag="gates")

                    for s in range(N_SLICES):
                        sl = slice(s * 512, (s + 1) * 512)
                        pt = ps.tile([P, 512], F32, tag="psum")
                        for kt in range(K_TILES):
                            nc.tensor.matmul(
                                pt[:],
                                lhsT=xhT[:, kt, :],
                                rhs=w16[:, kt, sl],
                                start=(kt == 0),
                                stop=(kt == K_TILES - 1),
                            )
                        nc.vector.tensor_tensor(gates[:, sl], pt[:], bb[:, sl], ADD)
                        nc.scalar.activation(
                            gates[:, sl], gates[:, sl], TANH if s >= 6 else SIG
                        )

                    i_g = gates[:, 0:H]
                    f_g = gates[:, H : 2 * H]
                    o_g = gates[:, 2 * H : 3 * H]
                    u_g = gates[:, 3 * H : 4 * H]

                    t1 = tp.tile([P, H], F32, tag="t1")
                    nc.vector.tensor_tensor(t1[:], f_g, cprev[:], MULT)
                    t2 = tp.tile([P, H], F32, tag="t2")
                    nc.vector.tensor_tensor(t2[:], i_g, u_g, MULT)
                    # c' overwrites the cprev slot; tanh(c') goes to t1's
                    # slot; h' to t2's slot.
                    nc.vector.tensor_tensor(cprev[:], t1[:], t2[:], ADD)
                    nc.scalar.activation(t1[:], cprev[:], TANH)
                    nc.vector.tensor_tensor(t2[:], o_g, t1[:], MULT)

                    nc.scalar.dma_start(hc_v[rows, 0, :], t2[:])
                    nc.scalar.dma_start(hc_v[rows, 1, :], cprev[:])

    nc.compile()
    return nc


_NC_CACHE: dict = {}


def _get_nc(b_local: int = B_LOCAL):
    if b_local not in _NC_CACHE:
        _NC_CACHE[b_local] = build_nc(b_local)
    return _NC_CACHE[b_local]


def make_in_maps(
    input, prev_h, prev_c,
    weight_xi, weight_hi, weight_xf, weight_hf,
    weight_xu, weight_hu, weight_xo, weight_ho,
    bias_i, bias_f, bias_o, bias_u,
):
    """Host-side shard/pack: batch split across cores, weights replicated.

    Gate column order [i | f | o | u]; K rows: x-weights then h-weights.
    Weights are pre-cast to fp16 here (the kernel computes in fp16 either
    way; pre-casting halves the weight HBM traffic)."""
    asnp = lambda a: np.ascontiguousarray(np.asarray(a, dtype=np.float32))
    w_cat = np.concatenate(
        [
            np.concatenate([asnp(weight_xi), asnp(weight_xf), asnp(weight_xo), asnp(weight_xu)], axis=1),
            np.concatenate([asnp(weight_hi), asnp(weight_hf), asnp(weight_ho), asnp(weight_hu)], axis=1),
        ],
        axis=0,
    ).astype(np.float16)
    b_cat = np.concatenate([asnp(bias_i), asnp(bias_f), asnp(bias_o), asnp(bias_u)], axis=0)
    x = asnp(input)
    hh = asnp(prev_h)
    cc = asnp(prev_c)
    in_maps = []
    for core in range(N_CORES):
        r = slice(core * B_LOCAL, (core + 1) * B_LOCAL)
        xhc = np.concatenate([x[r], hh[r], cc[r]], axis=0)
        in_maps.append({"xhc": xhc, "w": w_cat, "b": b_cat})
    return in_maps


def kernel(**inputs):
    nc = _get_nc()
    in_maps = make_in_maps(**inputs)
    res = run_bass_kernel_spmd(nc, in_maps, core_ids=list(range(N_CORES)))
    h_full = np.concatenate(
        [res.results[c]["hc_out"][:B_LOCAL] for c in range(N_CORES)], axis=0
    )
    c_full = np.concatenate(
        [res.results[c]["hc_out"][B_LOCAL:] for c in range(N_CORES)], axis=0
    )
    return (h_full, c_full)


if __name__ == "__main__":
    rng = np.random.default_rng(0)
    stdv = 1.0 / np.sqrt(H)
    ins = {
        "input": rng.standard_normal((B_FULL, IN), dtype=np.float32),
        "prev_h": rng.standard_normal((B_FULL, IN), dtype=np.float32),
        "prev_c": rng.standard_normal((B_FULL, IN), dtype=np.float32),
    }
    for nm in ["weight_xi", "weight_hi", "weight_xf", "weight_hf",
               "weight_xu", "weight_hu", "weight_xo", "weight_ho"]:
        ins[nm] = rng.uniform(-stdv, stdv, (IN, H)).astype(np.float32)
    for nm in ["bias_i", "bias_f", "bias_o", "bias_u"]:
        ins[nm] = rng.uniform(-stdv, stdv, (H,)).astype(np.float32)
    h, c = kernel(**ins)
    print("kernel ran:", h.shape, c.shape)


# revision 3
# speedup vs baseline: 41.2114x; 1.0660x over previous
"""LSTMCell Trainium2 kernel.

Full-input contract: kernel(**inputs) takes the complete (16384, 1024) fp32
tensors, shards the batch dim across 8 NeuronCores (data-parallel, weights
replicated), runs a Bass/Tile kernel per core, and gathers (h, c).

Per-core plan (B_local = 2048):
  - Inputs packed host-side: xhc = [x; prev_h; prev_c] rows ([3*B_local,
    1024] fp32), w = fp16 [2048, 4096] (x-weight rows then h-weight rows,
    gate cols [i|f|o|u]), b = fp32 [4096].  Output hc = [h'; c'] rows.
  - Weights are fetched as 16 per-k-tile HWDGE DMAs on the SCALAR ring
    (the sync ring carries the xbar transposes; HWDGE rings are FIFO, so
    weight streaming there would stall the next rep's first transpose)
    into two resident fp16 SBUF tiles [128, 8, 4096] (k on partitions,
    kt 0-7 / 8-15) -- the split halves what the per-rep refetch of each
    half must wait for, letting the first matmuls start early.
  - Per m-tile (128 batch rows): ONE gpsimd DMA loads the x+h rows with an
    fp32->fp16 cast; ONE HWDGE xbar-transpose instruction produces all 16
    k-partitioned tiles (xhT[p, kt, q] = xh16[q, kt*128+p]); 8 PSUM banks
    each accumulate preact[:, s*512:(s+1)*512] over the 16 k-tiles; DVE
    adds the partition-broadcast bias; ScalarE applies Sigmoid/Tanh;
    VectorE combines c' = f*c + i*u, h' = o*tanh(c'); h'/c' DMA out fp32.

reps > 1 wraps the whole body -- weight fetch included -- in a hardware
For_i that recomputes the same outputs.  test.py uses it to measure the
sustained per-execution time on hardware (a single execution cannot be
wall-clocked over the axon tunnel: ~70 ms dispatch latency vs ~0.5 ms of
device work).
"""

import sys

if "/opt/trn_rl_repo" not in sys.path:
    sys.path.insert(0, "/opt/trn_rl_repo")

import numpy as np

import concourse.bass as bass  # noqa: F401
import concourse.mybir as mybir
import concourse.tile as tile
from concourse import bacc
from concourse.bass_utils import run_bass_kernel_spmd

F32 = mybir.dt.float32
F16 = mybir.dt.float16

N_CORES = 8
B_FULL = 16384
IN = 1024
H = 1024
B_LOCAL = B_FULL // N_CORES  # 2048
P = 128
K_TILES = (IN + H) // P      # 16
N_TOTAL = 4 * H              # 4096 (gates i|f|o|u)
N_SLICES = N_TOTAL // 512    # 8
SIG = mybir.ActivationFunctionType.Sigmoid
TANH = mybir.ActivationFunctionType.Tanh
ADD = mybir.AluOpType.add
MULT = mybir.AluOpType.mult


class _NullCtx:
    def __enter__(self):
        return None

    def __exit__(self, *a):
        return False


def _maybe_for_i(tc, reps):
    return tc.For_i(0, reps, 1) if reps > 1 else _NullCtx()


def build_nc(b_local: int = B_LOCAL, reps: int = 1):
    m_tiles = b_local // P
    nc = bacc.Bacc("TRN2", target_bir_lowering=False, debug=False)

    xhc_d = nc.dram_tensor("xhc", [3 * b_local, IN], F32, kind="ExternalInput")
    w_d = nc.dram_tensor("w", [IN + H, N_TOTAL], F16, kind="ExternalInput")
    b_d = nc.dram_tensor("b", [N_TOTAL], F32, kind="ExternalInput")
    hc_d = nc.dram_tensor("hc_out", [2 * b_local, H], F32, kind="ExternalOutput")

    xhc_v = xhc_d.ap().rearrange("(pl b) f -> b pl f", pl=3)
    hc_v = hc_d.ap().rearrange("(pl b) f -> b pl f", pl=2)
    w_v = w_d.ap().rearrange("(kt p) g -> p kt g", kt=K_TILES)

    with tile.TileContext(nc) as tc:
        with (
            tc.tile_pool(name="wpool", bufs=1) as wpool,
            tc.tile_pool(name="const", bufs=1) as const,
        ):
            # Bias broadcast across partitions: [128, 4096] fp32.  The
            # staging tile lives in a pool that closes before the main
            # pools open (SBUF is tight: weights take 128KB/partition).
            bb = const.tile([P, N_TOTAL], F32)
            with tc.tile_pool(name="binit", bufs=1) as binit:
                b_sb = binit.tile([1, N_TOTAL], F32)
                nc.sync.dma_start(b_sb[:], b_d.ap().rearrange("(o n) -> o n", o=1))
                nc.gpsimd.partition_broadcast(bb[:], b_sb[:])

            # Resident fp16 weights, k on partitions, split into two
            # tiles (kt 0-7 / 8-15) so the per-rep refetch of each half only
            # waits on the matmuls that read that half.
            wA = wpool.tile([P, K_TILES // 2, N_TOTAL], F16, name="wA")
            wB = wpool.tile([P, K_TILES // 2, N_TOTAL], F16, name="wB")

            with (
                tc.tile_pool(name="stage", bufs=2) as stage,
                tc.tile_pool(name="xt", bufs=2) as xtp,
                tc.tile_pool(name="cin", bufs=2) as cin,
                tc.tile_pool(name="gate", bufs=2) as gp,
                tc.tile_pool(name="tmp", bufs=2) as tp,
                tc.tile_pool(name="ps", bufs=8, space="PSUM") as ps,
                _maybe_for_i(tc, reps),
            ):
                # Weight fetch (fp16, HWDGE, no cast) inside the reps loop
                # so each rep pays the full kernel's HBM traffic.  On the
                # scalar ring: the sync ring carries the xbar transposes and
                # HWDGE rings are FIFO, so w streaming there would stall the
                # next rep's first transpose (and with it the first matmul).
                for kt in range(K_TILES // 2):
                    nc.scalar.dma_start(wA[:, kt, :], w_v[:, kt, :])
                for kt in range(K_TILES // 2):
                    nc.scalar.dma_start(wB[:, kt, :], w_v[:, K_TILES // 2 + kt, :])

                for m in range(m_tiles):
                    rows = slice(m * P, (m + 1) * P)
                    # x+h rows for this m-tile in one DMA, fp32->fp16 cast.
                    xh16 = stage.tile([P, 2, IN], F16, tag="xh16")
                    nc.gpsimd.dma_start(xh16[:], xhc_v[rows, 0:2, :])
                    # All 16 k-tiles transposed in one xbar instruction:
                    # xhT[p, kt, q] = xh16[q, kt*128+p]
                    xhT = xtp.tile([P, K_TILES, P], F16, tag="xhT")
                    nc.sync.dma_start(xhT[:], xh16[:], transpose=True)

                    cprev = cin.tile([P, H], F32, tag="cprev")
                    nc.scalar.dma_start(cprev[:], xhc_v[rows, 2, :])

                    gates = gp.tile([P, N_TOTAL], F16, tag="gates")

                    for s in range(N_SLICES):
                        sl = slice(s * 512, (s + 1) * 512)
                        pt = ps.tile([P, 512], F32, tag="psum")
                        for kt in range(K_TILES):
                            wt = wA if kt < K_TILES // 2 else wB
                            nc.tensor.matmul(
                                pt[:],
                                lhsT=xhT[:, kt, :],
                                rhs=wt[:, kt % (K_TILES // 2), sl],
                                start=(kt == 0),
                                stop=(kt == K_TILES - 1),
                            )
                        nc.vector.tensor_tensor(gates[:, sl], pt[:], bb[:, sl], ADD)
                        nc.scalar.activation(
                            gates[:, sl], gates[:, sl], TANH if s >= 6 else SIG
                        )

                    i_g = gates[:, 0:H]
                    f_g = gates[:, H : 2 * H]
                    o_g = gates[:, 2 * H : 3 * H]
                    u_g = gates[:, 3 * H : 4 * H]

                    t1 = tp.tile([P, H], F32, tag="t1")
                    nc.vector.tensor_tensor(t1[:], f_g, cprev[:], MULT)
                    t2 = tp.tile([P, H], F32, tag="t2")
                    nc.vector.tensor_tensor(t2[:], i_g, u_g, MULT)
                    # c' overwrites the cprev slot; tanh(c') goes to t1's
                    # slot; h' to t2's slot.
                    nc.vector.tensor_tensor(cprev[:], t1[:], t2[:], ADD)
                    nc.scalar.activation(t1[:], cprev[:], TANH)
                    nc.vector.tensor_tensor(t2[:], o_g, t1[:], MULT)

                    nc.scalar.dma_start(hc_v[rows, 0, :], t2[:])
                    nc.scalar.dma_start(hc_v[rows, 1, :], cprev[:])

    nc.compile()
    return nc


_NC_CACHE: dict = {}


def _get_nc(b_local: int = B_LOCAL):
    if b_local not in _NC_CACHE:
        _NC_CACHE[b_local] = build_nc(b_local)
    return _NC_CACHE[b_local]


def make_in_maps(
    input, prev_h, prev_c,
    weight_xi, weight_hi, weight_xf, weight_hf,
    weight_xu, weight_hu, weight_xo, weight_ho,
    bias_i, bias_f, bias_o, bias_u,
):
    """Host-side shard/pack: batch split across cores, weights replicated.

    Gate column order [i | f | o | u]; K rows: x-weights then h-weights.
    Weights are pre-cast to fp16 here (the kernel computes in fp16 either
    way; pre-casting halves the weight HBM traffic)."""
    asnp = lambda a: np.ascontiguousarray(np.asarray(a, dtype=np.float32))
    w_cat = np.concatenate(
        [
            np.concatenate([asnp(weight_xi), asnp(weight_xf), asnp(weight_xo), asnp(weight_xu)], axis=1),
            np.concatenate([asnp(weight_hi), asnp(weight_hf), asnp(weight_ho), asnp(weight_hu)], axis=1),
        ],
        axis=0,
    ).astype(np.float16)
    b_cat = np.concatenate([asnp(bias_i), asnp(bias_f), asnp(bias_o), asnp(bias_u)], axis=0)
    x = asnp(input)
    hh = asnp(prev_h)
    cc = asnp(prev_c)
    in_maps = []
    for core in range(N_CORES):
        r = slice(core * B_LOCAL, (core + 1) * B_LOCAL)
        xhc = np.concatenate([x[r], hh[r], cc[r]], axis=0)
        in_maps.append({"xhc": xhc, "w": w_cat, "b": b_cat})
    return in_maps


def kernel(**inputs):
    nc = _get_nc()
    in_maps = make_in_maps(**inputs)
    res = run_bass_kernel_spmd(nc, in_maps, core_ids=list(range(N_CORES)))
    h_full = np.concatenate(
        [res.results[c]["hc_out"][:B_LOCAL] for c in range(N_CORES)], axis=0
    )
    c_full = np.concatenate(
        [res.results[c]["hc_out"][B_LOCAL:] for c in range(N_CORES)], axis=0
    )
    return (h_full, c_full)


if __name__ == "__main__":
    rng = np.random.default_rng(0)
    stdv = 1.0 / np.sqrt(H)
    ins = {
        "input": rng.standard_normal((B_FULL, IN), dtype=np.float32),
        "prev_h": rng.standard_normal((B_FULL, IN), dtype=np.float32),
        "prev_c": rng.standard_normal((B_FULL, IN), dtype=np.float32),
    }
    for nm in ["weight_xi", "weight_hi", "weight_xf", "weight_hf",
               "weight_xu", "weight_hu", "weight_xo", "weight_ho"]:
        ins[nm] = rng.uniform(-stdv, stdv, (IN, H)).astype(np.float32)
    for nm in ["bias_i", "bias_f", "bias_o", "bias_u"]:
        ins[nm] = rng.uniform(-stdv, stdv, (H,)).astype(np.float32)
    h, c = kernel(**ins)
    print("kernel ran:", h.shape, c.shape)
